# revision 1
# baseline (speedup 1.0000x reference)
"""Multi-head attention (RoPE, causal) Trainium2 Bass kernel.

Sharding (8 cores): data-parallel over batch (4) x tensor-parallel over
heads (16 -> 2 groups of 8).  Core c handles batch c//2 and head group
c%2.  Attention is fully head-local; the out-projection partial sums of
the two head groups of each batch are added on the host.

Per-core device kernel (sizes hardcoded for b=4, n=2048, hidden=1024,
h=16, d=64):
  - QKV projections as fp32r matmuls; x and W arrive host-pre-transposed
    (c-major) so no on-device transposes are needed.
  - RoPE applied in head-transposed layout via a +-1 pair-permutation
    matmul (PE) plus q*cos + rot(q)*sin on DVE.  The softmax scale 1/8 is
    folded into the exp activation's scale.
  - Scores computed transposed, s_T[k, q] (128 k-partitions x 512 q),
    with the two heads of a pair row-tiled onto the PE array at
    tile_position (0,0)/(64,0).  Causal masking via tensor_mask_reduce
    (masked scores -> -FLT_MAX), exp on ScalarE -> bf16 p.
  - AV uses v augmented with a ones column (M=65): PSUM row 64 accumulates
    the softmax denominator for free.  Normalization multiplies by a
    DMA-broadcast reciprocal.
  - Causality lets attention and the out-projection stream behind the
    projections strip by strip; q/ao live only for the current strip.
"""

import numpy as np
import ml_dtypes

import concourse.bass as bass
import concourse.mybir as mybir
from concourse import bacc
from concourse.tile import TileContext
from concourse.bass_utils import run_bass_kernel_spmd

# ---------------------------------------------------------------- constants
B, N, HID = 4, 2048, 1024
H = 16
D = HID // H                     # 64
NCORES = 8
GROUPS = NCORES // B             # 2 head groups
HPG = H // GROUPS                # 8 heads per core
HD = HPG * D                     # 512 local head dims
PAIRS = HPG // 2                 # 4 head pairs per core
ROPE_THETA = 10000.0
SCALE = 0.125                    # 1/sqrt(d)

P = 128
CC = HID // P                    # 8 contraction chunks for projections
ICH = 512                        # projection i-chunk (moving free dim)
QCH = 512                        # attention q-chunk
KCH = 128                        # attention k-chunk
NQC = N // QCH                   # 4
NKC = N // KCH                   # 16

F32 = mybir.dt.float32
F32R = mybir.dt.float32r
BF16 = mybir.dt.bfloat16
BF16NP = ml_dtypes.bfloat16

_NC_CACHE = {}
DEBUG = False
MAX_STRIPS = None
REPEAT = 1


# ---------------------------------------------------------------- host prep
def _allow_matrix(is_causal, start_pos):
    i = np.arange(N)[:, None]    # query index
    j = np.arange(N)[None, :]    # key index
    if is_causal:
        return (j < start_pos) | ((i >= start_pos) & (i >= j))
    return np.ones((N, N), dtype=bool)


def _block_plan(is_causal, start_pos):
    """Classify each (qc, kc) score block; return plan + mask-start table.

    plan[(qc, kc)] is 'skip', 'full', or ('partial', variant_idx).
    Variant v is a [128] float32 vector: first valid q column (within the
    512-wide q chunk) for each k partition row.
    """
    allow = _allow_matrix(is_causal, start_pos)
    plan = {}
    variants = []
    vkeys = {}
    for qc in range(NQC):
        for kc in range(NKC):
            blk = allow[qc * QCH:(qc + 1) * QCH, kc * KCH:(kc + 1) * KCH]
            if not blk.any():
                plan[(qc, kc)] = ("skip", None)
                continue
            if blk.all():
                plan[(qc, kc)] = ("full", None)
                continue
            bT = blk.T               # [128 k, 512 q]
            start = np.argmax(bT, axis=1)
            for r in range(KCH):
                if not bT[r].any():
                    raise NotImplementedError("empty k-row in partial block")
                s = start[r]
                if not bT[r, s:].all() or bT[r, :s].any():
                    raise NotImplementedError("non-suffix mask row")
            key = start.tobytes()
            if key not in vkeys:
                vkeys[key] = len(variants)
                variants.append(start.astype(np.float32))
            plan[(qc, kc)] = ("partial", vkeys[key])
    if not variants:
        variants.append(np.zeros(KCH, dtype=np.float32))
    # variant v -> [128, QCH] 0/1 mask block: valid iff q_local >= start[k]
    q = np.arange(QCH)[None, :]
    blocks = [(q >= v[:, None]).astype(np.float32) for v in variants]
    masks = np.concatenate(blocks, axis=1)        # [128, V*QCH]
    return plan, masks


def _rope_tables():
    inv_freq = 1.0 / (ROPE_THETA ** (np.arange(0, D, 2, dtype=np.float64) / D))
    t = np.arange(N, dtype=np.float64)
    freqs = t[:, None] * inv_freq[None, :]        # [N, 32]
    freqs = np.repeat(freqs, 2, axis=1)           # [N, 64]
    cos = np.cos(freqs).T.astype(np.float32)      # [64, N]
    sin = np.sin(freqs).T.astype(np.float32)
    # duplicate rows so both heads of a pair (partitions 0:64 / 64:128)
    # see the same table at matching partition base
    cos2 = np.concatenate([cos, cos], axis=0)     # [128, N]
    sin2 = np.concatenate([sin, sin], axis=0)
    return np.ascontiguousarray(cos2), np.ascontiguousarray(sin2)


def _perm_matrix():
    # rot = PM @ q  with rot[2r] = -q[2r+1], rot[2r+1] = q[2r].
    # matmul computes lhsT.T @ rhs, so pass PM.T.
    pm = np.zeros((P, P), dtype=np.float32)
    for r in range(P // 2):
        pm[2 * r, 2 * r + 1] = -1.0
        pm[2 * r + 1, 2 * r] = 1.0
    return np.ascontiguousarray(pm.T)


# ---------------------------------------------------------------- device IR
def _build_nc(is_causal, start_pos):
    plan, masks_np = _block_plan(is_causal, start_pos)
    nvar = masks_np.shape[1] // QCH
    streaming = bool(is_causal)

    nc = bacc.Bacc("TRN2", target_bir_lowering=False, debug=False)

    xqT = nc.declare_dram_parameter("xqT", [HID, N], F32R, isOutput=False).ap()
    xkT = nc.declare_dram_parameter("xkT", [HID, N], F32R, isOutput=False).ap()
    xvT = nc.declare_dram_parameter("xvT", [HID, N], BF16, isOutput=False).ap()
    wqT = nc.declare_dram_parameter("wqT", [HID, HD], F32R, isOutput=False).ap()
    wkT = nc.declare_dram_parameter("wkT", [HID, HD], F32R, isOutput=False).ap()
    wvT = nc.declare_dram_parameter("wvT", [HID, HD], BF16, isOutput=False).ap()
    woT = nc.declare_dram_parameter("woT", [HD, HID], BF16, isOutput=False).ap()
    cos_d = nc.declare_dram_parameter("cos", [P, N], F32, isOutput=False).ap()
    sin_d = nc.declare_dram_parameter("sin", [P, N], F32, isOutput=False).ap()
    pm_d = nc.declare_dram_parameter("pm", [P, P], F32R, isOutput=False).ap()
    msk_d = nc.declare_dram_parameter("masks", [P, nvar * QCH], BF16,
                                      isOutput=False).ap()
    kpad_d = nc.declare_dram_parameter("kpad", [P, NKC], BF16, isOutput=False).ap()
    bc1_d = nc.declare_dram_parameter("bc1", [1, P], F32R, isOutput=False).ap()
    y = nc.declare_dram_parameter("y", [N, HID], F32, isOutput=True).ap()
    dbg = {}
    if DEBUG:
        dbg["dn"] = nc.declare_dram_parameter("dbg_dn", [NQC, 2, QCH], F32, isOutput=True).ap()
        dbg["rbc"] = nc.declare_dram_parameter("dbg_rbc", [NQC, P, QCH], F32, isOutput=True).ap()
        dbg["ao"] = nc.declare_dram_parameter("dbg_ao", [NQC, P, QCH], F32, isOutput=True).ap()
        dbg["p"] = nc.declare_dram_parameter("dbg_p", [NQC, P, QCH], BF16, isOutput=True).ap()
        dbg["s"] = nc.declare_dram_parameter("dbg_s", [NQC, P, QCH], F32, isOutput=True).ap()
        dbg["av"] = nc.declare_dram_parameter("dbg_av", [NQC, D + 1, QCH], F32, isOutput=True).ap()
        dbg["v"] = nc.declare_dram_parameter("dbg_v", [P, NKC * (D + 1)], BF16, isOutput=True).ap()
        dbg["av2"] = nc.declare_dram_parameter("dbg_av2", [PAIRS, NQC, 2, D + 1, QCH], F32, isOutput=True).ap()
        dbg["ao2"] = nc.declare_dram_parameter("dbg_ao2", [PAIRS, NQC, P, QCH], BF16, isOutput=True).ap()
        dbg["vps"] = nc.declare_dram_parameter("dbg_vps", [N // P, P, HD], F32, isOutput=True).ap()

    with TileContext(nc) as tc:
        with (
            tc.tile_pool(name="const", bufs=1) as const,
            tc.tile_pool(name="persist", bufs=1) as persist,
            tc.tile_pool(name="xstrip", bufs=2 if not DEBUG else 1) as xpool,
            tc.tile_pool(name="qpool", bufs=2) as qpool,
            tc.tile_pool(name="aopool", bufs=2) as aopool,
            tc.tile_pool(name="work", bufs=2) as work,
            tc.tile_pool(name="dbgpool", bufs=1) as dbgpool,
            tc.tile_pool(name="ppool", bufs=6) as ppool,
            tc.tile_pool(name="psmm", bufs=5, space="PSUM") as psmm,
            tc.tile_pool(name="ps2", bufs=3, space="PSUM") as ps2,
        ):
            # ---------------- constants / tables
            cost = const.tile([P, N], F32, tag="cos", name="cos")
            sint = const.tile([P, N], F32, tag="sin", name="sin")
            nc.sync.dma_start(out=cost, in_=cos_d)
            nc.sync.dma_start(out=sint, in_=sin_d)
            pmt = const.tile([P, P], F32R, tag="pm", name="pm")
            nc.sync.dma_start(out=pmt, in_=pm_d)
            mskt = const.tile([P, nvar * QCH], BF16, tag="masks", name="mskt")
            nc.sync.dma_start(out=mskt, in_=msk_d)
            bc1 = const.tile([1, P], F32R, tag="bc1", name="bc1")
            nc.sync.dma_start(out=bc1, in_=bc1_d)
            kpad = const.tile([P, NKC], BF16, tag="kpad", name="kpad")
            nc.sync.dma_start(out=kpad, in_=kpad_d)

            # ---------------- persistent activations
            kT = [persist.tile([P, N], F32R, tag=f"kT{p}", name=f"kT{p}")
                  for p in range(PAIRS)]
            vt = [persist.tile([P, NKC * (D + 1)], BF16, tag=f"v{h}",
                               name=f"v{h}") for h in range(HPG)]
            if streaming:
                qT = None
            else:
                qT = [persist.tile([P, N], F32R, tag=f"qT{p}", name=f"qT{p}")
                      for p in range(PAIRS)]

            # ---------------- weights
            wq = const.tile([P, CC, HD], F32R, tag="wq", name="wq")
            nc.sync.dma_start(out=wq, in_=wqT.rearrange("(cc p) m -> p cc m", p=P))
            wk = const.tile([P, CC, HD], F32R, tag="wk", name="wk")
            nc.sync.dma_start(out=wk, in_=wkT.rearrange("(cc p) m -> p cc m", p=P))
            wv = const.tile([P, CC, HD], BF16, tag="wv", name="wv")
            nc.sync.dma_start(out=wv, in_=wvT.rearrange("(cc p) m -> p cc m", p=P))
            wo = const.tile([P, PAIRS, HID], BF16, tag="wo", name="wo")
            nc.sync.dma_start(out=wo, in_=woT.rearrange("(jc p) o -> p jc o", p=P))

            def proj_qk_strip(x_dram, w_sb, ic, dst_of_mc):
                """Project one 512-wide strip of q or k (all pairs) + RoPE."""
                strip = xpool.tile([P, CC, ICH], F32R, tag="xstrip",
                                   name="strip")
                nc.sync.dma_start(
                    out=strip,
                    in_=x_dram.rearrange("(cc p) n -> p cc n", p=P)[
                        :, :, ic * ICH:(ic + 1) * ICH],
                )
                for mc in range(PAIRS):
                    ps = psmm.tile([P, ICH], F32, tag="mm", name="pjmm")
                    for cc in range(CC):
                        nc.tensor.matmul(
                            ps,
                            lhsT=w_sb[:, cc, mc * P:(mc + 1) * P],
                            rhs=strip[:, cc, :],
                            start=(cc == 0),
                            stop=(cc == CC - 1),
                        )
                    raw = work.tile([P, ICH], F32R, tag="raw", name="raw")
                    nc.vector.tensor_copy(out=raw, in_=ps)
                    rps = ps2.tile([P, ICH], F32, tag="ps2", name="rotps")
                    nc.tensor.matmul(rps, lhsT=pmt, rhs=raw,
                                     start=True, stop=True)
                    dsth = dst_of_mc(mc)
                    tmp = work.tile([P, ICH], F32, tag="ropetmp", name="tmp")
                    nc.vector.tensor_mul(tmp, rps, sint[:, ic * ICH:(ic + 1) * ICH])
                    nc.vector.tensor_mul(dsth, raw, cost[:, ic * ICH:(ic + 1) * ICH])
                    nc.vector.tensor_add(dsth, dsth, tmp)

            def proj_v_strip(ic4):
                """Project one 512-wide strip of v into vt (+pad, +ones)."""
                strip = xpool.tile([P, CC, ICH], BF16, tag="xstrip",
                                   name="vstrip")
                nc.sync.dma_start(
                    out=strip,
                    in_=xvT.rearrange("(cc p) n -> p cc n", p=P)[
                        :, :, ic4 * ICH:(ic4 + 1) * ICH],
                )
                for sub in range(ICH // P):
                    ic = ic4 * (ICH // P) + sub
                    ps = psmm.tile([P, HD], F32, tag="mm", name="pvmm")
                    for cc in range(CC):
                        nc.tensor.matmul(
                            ps,
                            lhsT=strip[:, cc, sub * P:(sub + 1) * P],
                            rhs=wv[:, cc, :],
                            start=(cc == 0),
                            stop=(cc == CC - 1),
                        )
                    padb = kpad[:, ic:ic + 1]
                    for h in range(HPG):
                        base = ic * (D + 1)
                        nc.vector.tensor_mul(
                            vt[h][:, base:base + D],
                            ps[:, h * D:(h + 1) * D],
                            padb.to_broadcast([P, D]),
                        )
                        nc.vector.tensor_copy(
                            out=vt[h][:, base + D:base + D + 1],
                            in_=padb,
                        )

            def attn_block(pp, qc, q_tile, ao_tile):
                """Attention for head pair pp over q chunk qc.

                q_tile: [128, 512] f32r (this qc's roped q for the pair)
                ao_tile: [128, 512] bf16 output (normalized attn @ v)
                """
                kcs = [kc for kc in range(NKC) if plan[(qc, kc)][0] != "skip"]
                if not kcs:
                    return
                avt = ps2.tile([P, QCH], F32, tag="ps2", name="avA")
                avt2 = ps2.tile([P, QCH], F32, tag="ps2", name="avB")
                avs = (avt[0:D + 1, :], avt2[0:D + 1, :])
                for idx, kc in enumerate(kcs):
                    ksl = slice(kc * KCH, (kc + 1) * KCH)
                    first, last = idx == 0, idx == len(kcs) - 1
                    kind, var = plan[(qc, kc)]
                    for hh in range(2):
                        base = hh * D
                        sp = psmm.tile([P, QCH], F32, tag="mm", name="smm")
                        nc.tensor.matmul(
                            sp,
                            lhsT=kT[pp][base:base + D, ksl],
                            rhs=q_tile[base:base + D, :],
                            start=True, stop=True,
                            tile_position=(base, 0),
                        )
                        pt = ppool.tile([P, QCH], BF16, tag="p", name="p")
                        nc.scalar.activation(
                            pt, sp, mybir.ActivationFunctionType.Exp,
                            scale=SCALE)
                        if kind == "partial":
                            nc.vector.tensor_mul(
                                pt, pt,
                                mskt[:, var * QCH:(var + 1) * QCH])
                        if DEBUG and pp == 0 and hh == 0 and kc == 0:
                            st = dbgpool.tile([P, QCH], F32, tag="dbgs", name="dbgs")
                            nc.vector.tensor_copy(out=st, in_=sp)
                            nc.sync.dma_start(out=dbg["s"][qc], in_=st)
                            nc.sync.dma_start(out=dbg["p"][qc], in_=pt)
                        nc.tensor.matmul(
                            avs[hh],
                            lhsT=vt[2 * pp + hh][
                                :, kc * (D + 1):(kc + 1) * (D + 1)],
                            rhs=pt,
                            start=first, stop=last,
                        )
                # normalize: broadcast denominators via K=1 ones matmul,
                # reciprocal on DVE, multiply into ao
                for hh in range(2):
                    dn = work.tile([1, QCH], F32R, tag=f"dn{hh}", name="dn")
                    nc.vector.tensor_copy(out=dn, in_=avs[hh][D:D + 1, :])
                    bc = psmm.tile([P, QCH], F32, tag="mm", name="bcps")
                    nc.tensor.matmul(bc[0:D, :], lhsT=bc1[:, 0:D], rhs=dn,
                                     start=True, stop=True)
                    rbch = work.tile([D, QCH], F32, tag=f"rbc{hh}",
                                     name="rbch")
                    nc.vector.reciprocal(rbch, bc[0:D, :])
                    nc.vector.tensor_mul(ao_tile[hh * D:(hh + 1) * D, :],
                                         avs[hh][0:D, :], rbch)
                if DEBUG:
                    nc.sync.dma_start(out=dbg["ao2"][pp, qc], in_=ao_tile)


            def outproj(ic, ao_of_pp):
                isl = slice((ic % (ICH // P)) * P, (ic % (ICH // P)) * P + P)
                for oc in range(HID // 512):
                    osl = slice(oc * 512, (oc + 1) * 512)
                    ps = psmm.tile([P, 512], F32, tag="mm", name="yps")
                    for pp in range(PAIRS):
                        nc.tensor.matmul(
                            ps,
                            lhsT=ao_of_pp(pp)[:, isl],
                            rhs=wo[:, pp, osl],
                            start=(pp == 0), stop=(pp == PAIRS - 1),
                        )
                    yt = work.tile([P, 512], F32, tag="yout", name="yt")
                    nc.vector.tensor_copy(out=yt, in_=ps)
                    nc.sync.dma_start(
                        out=y[ic * P:(ic + 1) * P, osl], in_=yt)

            reps = range(REPEAT)
            if streaming:
                nstrips = N // ICH if MAX_STRIPS is None else MAX_STRIPS
                for _rep in reps:
                 for ic in range(nstrips):
                     proj_v_strip(ic)
                     q_tiles = [qpool.tile([P, ICH], F32R, tag=f"qs{mc}",
                                           name=f"qs{mc}")
                                for mc in range(PAIRS)]
                     proj_qk_strip(xqT, wq, ic, lambda mc: q_tiles[mc])
                     proj_qk_strip(xkT, wk, ic,
                                   lambda mc: kT[mc][:, ic * ICH:(ic + 1) * ICH])
                     ao_tiles = [aopool.tile([P, QCH], BF16, tag=f"aos{pp}",
                                             name=f"aos{pp}")
                                 for pp in range(PAIRS)]
                     for pp in range(PAIRS):
                         attn_block(pp, ic, q_tiles[pp], ao_tiles[pp])
                     for sub in range(ICH // P):
                         outproj(ic * (ICH // P) + sub,
                                 lambda pp: ao_tiles[pp])
            else:
              for _rep in reps:
                for ic in range(N // ICH):
                    proj_v_strip(ic)
                    proj_qk_strip(xqT, wq, ic,
                                  lambda mc: qT[mc][:, ic * ICH:(ic + 1) * ICH])
                    proj_qk_strip(xkT, wk, ic,
                                  lambda mc: kT[mc][:, ic * ICH:(ic + 1) * ICH])
                for qc in range(NQC):  # noqa
                    ao_tiles = [aopool.tile([P, QCH], BF16, tag=f"aos{pp}",
                                            name=f"aos{pp}")
                                for pp in range(PAIRS)]
                    for pp in range(PAIRS):
                        attn_block(pp, qc, qT[pp][:, qc * QCH:(qc + 1) * QCH],
                                   ao_tiles[pp])
                    for sub in range(QCH // P):
                        outproj(qc * (QCH // P) + sub,
                                lambda pp: ao_tiles[pp])


    nc.compile()
    return nc, masks_np


def _get_nc(is_causal, start_pos):
    key = (bool(is_causal), int(start_pos), REPEAT, MAX_STRIPS, DEBUG)
    if key not in _NC_CACHE:
        _NC_CACHE[key] = _build_nc(bool(is_causal), int(start_pos))
    return _NC_CACHE[key]


# ---------------------------------------------------------------- entry
def kernel(x_q, x_k, x_v, W_q, W_k, W_v, W_out, padding_mask, is_causal,
           start_pos):
    x_q = np.asarray(x_q, dtype=np.float32)
    x_k = np.asarray(x_k, dtype=np.float32)
    x_v = np.asarray(x_v, dtype=np.float32)
    W_q = np.asarray(W_q, dtype=np.float32)
    W_k = np.asarray(W_k, dtype=np.float32)
    W_v = np.asarray(W_v, dtype=np.float32)
    W_out = np.asarray(W_out, dtype=np.float32)
    padding_mask = np.asarray(padding_mask).astype(bool)
    is_causal = int(np.asarray(is_causal))
    start_pos = int(np.asarray(start_pos))

    nc, masks = _get_nc(is_causal, start_pos)

    cos2, sin2 = _rope_tables()
    pm = _perm_matrix()

    in_maps = []
    for c in range(NCORES):
        bi, hg = divmod(c, GROUPS)
        hs = hg * HD
        kpad = np.ascontiguousarray(
            padding_mask[bi].astype(np.float32).reshape(NKC, P).T
        ).astype(BF16NP)
        in_maps.append({
            "xqT": np.ascontiguousarray(x_q[bi].T),
            "xkT": np.ascontiguousarray(x_k[bi].T),
            "xvT": np.ascontiguousarray(x_v[bi].T).astype(BF16NP),
            "wqT": np.ascontiguousarray(W_q[hs:hs + HD].T),
            "wkT": np.ascontiguousarray(W_k[hs:hs + HD].T),
            "wvT": np.ascontiguousarray(W_v[hs:hs + HD].T).astype(BF16NP),
            "woT": np.ascontiguousarray(W_out[:, hs:hs + HD].T).astype(BF16NP),
            "cos": cos2,
            "sin": sin2,
            "pm": pm,
            "masks": masks.astype(BF16NP),
            "kpad": kpad,
            "bc1": np.ones((1, P), dtype=np.float32),
        })

    res = run_bass_kernel_spmd(nc, in_maps, list(range(NCORES)))
    out = np.empty((B, N, HID), dtype=np.float32)
    for bi in range(B):
        out[bi] = res.results[GROUPS * bi]["y"]
        for g in range(1, GROUPS):
            out[bi] += res.results[GROUPS * bi + g]["y"]
    return out

